# revision 3
# baseline (speedup 1.0000x reference)
"""Trainium2 Bass kernel for nn_GCNLayer_36962488549946 — v3.

Per batch element b and point n: knn (K=8, incl. self) by pairwise squared
distance, gather neighbor features, feat = [neigh - x, x] @ W^T, BatchNorm
(inference), LeakyReLU(0.2), max over the 8 neighbors.

Algebraic refactor (exact: bn scale > 0 and LeakyReLU monotone, so max over k
commutes with the per-channel affine + activation):
    out[n,u] = LRelu( max_k A[idx[n,k], u]  +  Bv[n, u] )
    A  = x @ (inv*W1)^T                 W1 = W[:, :C], inv = gamma*rsqrt(var+eps)
    Bv = x @ (inv*(W2-W1))^T + shift    W2 = W[:, C:], shift = beta - mean*inv

v3 design:
- Scoring is ONE fp16 matmul (K=67): score = h_r.h_m + sq_h(m) + sq_l(m)
  + rr(r); sq = -|x_m|^2/2 split hi/lo fp16; rr = (-1/2-GAMMA)*|x_r|^2 - C0
  is row-constant (ranking-neutral). fp16 input rounding flips ~119/32768
  rows' neighbor sets -> rel err ~1e-2 (tolerance 2e-2).
- A/Bv GEMMs single-term fp16 (adds rel err ~2.4e-4).
- dist fp32 in SBUF; MAX8 + FIND_INDEX = exact top-8 of the fp16-rounded
  scores (HW resolves duplicate values to distinct successive indices).
- Software-pipelined main loop: tile t's post-gather ops (k-max reduce,
  +Bv, LeakyReLU, store) are emitted AFTER tile t+1's MAX8/FIND_INDEX so
  the Vector queue never stalls on gather completion.

Sharding: 8 cores; core c handles batch b = c // 2, row half (c % 2).
No cross-core communication; host reassembles.
"""

import numpy as np

import concourse.bacc as bacc
import concourse.bass as bass
import concourse.mybir as mybir
import concourse.tile as tile
from concourse.bass import ts
from concourse.bass_utils import run_bass_kernel_spmd
from concourse.masks import make_identity

F32 = mybir.dt.float32
F16 = mybir.dt.float16
U32 = mybir.dt.uint32
AF = mybir.ActivationFunctionType
ALU = mybir.AluOpType

K = 8
BN_EPS = 1e-3
NEG_SLOPE = 0.2
GAMMA = -0.2684338515047087
C0 = -17.232328969428075

B_FULL, N_FULL, C_FULL, U_FULL = 4, 8192, 64, 64
N_CORES = 8
KC = 67  # contract rows: 64 h + sq_h + sq_l + (ones|rr)


def build_nc(N=N_FULL, ROWS=N_FULL // 2, C=C_FULL, U=U_FULL, CHUNK=512):
    assert N % 128 == 0 and ROWS % 128 == 0 and N % CHUNK == 0 and CHUNK <= 512
    assert C == 64 and U == 64
    n_mt = N // 128
    n_rt = ROWS // 128
    n_ck = N // CHUNK

    nc = bacc.Bacc(trn_type="TRN2")

    xin = nc.declare_dram_parameter("xin", [N, C], F32, isOutput=False)
    xrows = nc.declare_dram_parameter("xrows", [ROWS, C], F32, isOutput=False)
    Wp = nc.declare_dram_parameter("W", [U, 2 * C], F32, isOutput=False)
    bn_gamma = nc.declare_dram_parameter("bn_gamma", [U], F32, isOutput=False)
    bn_beta = nc.declare_dram_parameter("bn_beta", [U], F32, isOutput=False)
    bn_mean = nc.declare_dram_parameter("bn_mean", [U], F32, isOutput=False)
    bn_var = nc.declare_dram_parameter("bn_var", [U], F32, isOutput=False)
    out = nc.declare_dram_parameter("out", [ROWS, U], F32, isOutput=True)

    A_dram = nc.dram_tensor("A_dram", [N, U], F32)

    with tile.TileContext(nc) as tc:
        with (
            tc.tile_pool(name="big", bufs=1) as big,
            tc.tile_pool(name="work", bufs=3) as work,
            tc.tile_pool(name="distp", bufs=2) as distp,
            tc.tile_pool(name="psD", bufs=4, space="PSUM") as psD,
            tc.tile_pool(name="psT", bufs=2, space="PSUM") as psT,
            tc.tile_pool(name="psA", bufs=2, space="PSUM") as psA,
        ):
            id128 = big.tile([128, 128], F32)
            make_identity(nc, id128[:])
            id64 = big.tile([64, 64], F32)
            make_identity(nc, id64[:])

            # ---------- BN inv in [u,1] layout ----------
            g64 = big.tile([64, 1], F32)
            nc.sync.dma_start(out=g64[:], in_=bn_gamma[:, None])
            v64 = big.tile([64, 1], F32)
            nc.sync.dma_start(out=v64[:], in_=bn_var[:, None])
            inv64 = big.tile([64, 1], F32)
            nc.vector.tensor_scalar(
                out=inv64[:], in0=v64[:], scalar1=BN_EPS, scalar2=None, op0=ALU.add
            )
            nc.scalar.sqrt(out=inv64[:], in_=inv64[:])
            nc.vector.reciprocal(out=inv64[:], in_=inv64[:])
            nc.vector.tensor_mul(out=inv64[:], in0=inv64[:], in1=g64[:])

            # ---------- shift = beta - mean*inv ----------
            b1 = big.tile([1, U], F32)
            nc.sync.dma_start(out=b1[:], in_=bn_beta[None, :])
            m1 = big.tile([1, U], F32)
            nc.sync.dma_start(out=m1[:], in_=bn_mean[None, :])
            g1 = big.tile([1, U], F32)
            nc.sync.dma_start(out=g1[:], in_=bn_gamma[None, :])
            v1 = big.tile([1, U], F32)
            nc.sync.dma_start(out=v1[:], in_=bn_var[None, :])
            t1 = big.tile([1, U], F32)
            nc.vector.tensor_scalar(
                out=t1[:], in0=v1[:], scalar1=BN_EPS, scalar2=None, op0=ALU.add
            )
            nc.scalar.sqrt(out=t1[:], in_=t1[:])
            nc.vector.reciprocal(out=t1[:], in_=t1[:])
            nc.vector.tensor_mul(out=t1[:], in0=t1[:], in1=g1[:])
            nc.vector.tensor_mul(out=m1[:], in0=m1[:], in1=t1[:])
            shift1 = big.tile([1, U], F32)
            nc.vector.tensor_sub(out=shift1[:], in0=b1[:], in1=m1[:])

            # ---------- scaled weights, transposed, fp16 ----------
            W_sb = big.tile([64, 2 * C], F32)
            nc.sync.dma_start(out=W_sb[:], in_=Wp[:, :])
            Wsc = big.tile([64, 2 * C], F32)
            nc.vector.tensor_scalar(
                out=Wsc[:], in0=W_sb[:], scalar1=inv64[:, 0:1], scalar2=None,
                op0=ALU.mult,
            )
            wt_ps = psA.tile([2 * C, U], F32, tag="small")
            nc.tensor.transpose(out=wt_ps[:], in_=Wsc[:], identity=id64[:])
            ra16 = big.tile([64, U], F16)  # W1'^T fp16
            nc.scalar.copy(out=ra16[:], in_=wt_ps[0:C, :])
            w1t = big.tile([64, U], F32)
            nc.vector.tensor_copy(out=w1t[:], in_=wt_ps[0:C, :])
            wdt = big.tile([64, U], F32)  # (W2'-W1')^T
            nc.vector.tensor_sub(out=wdt[:], in0=wt_ps[C:2 * C, :], in1=w1t[:])
            rw16 = big.tile([KC, U], F16)  # [ (W2'-W1')^T ; shift ; 0 ; 0 ]
            nc.vector.memset(rw16[:], 0.0)
            nc.scalar.copy(out=rw16[0:C, :], in_=wdt[:])
            nc.scalar.copy(out=rw16[64:65, :], in_=shift1[:])

            # ---------- xin -> H [KC, N] fp16 + A table ----------
            # pair-loads: [128, 128] = two row-tiles side by side -> one
            # transpose; Vector issues the loads + A copies (idle in preamble)
            H = big.tile([KC, N], F16)
            nc.vector.memset(H[64:KC, :], 1.0)  # row 66 stays ones
            sqmat = big.tile([128, n_mt], F32)
            for i2 in range(n_mt // 2):
                xt2 = work.tile([128, 2, C], F32, tag="xload")
                nc.sync.dma_start(
                    out=xt2[:],
                    in_=xin[ts(i2, 256), :].rearrange("(a p) c -> p a c", p=128),
                )
                tp2 = psT.tile([128, 128], F32, tag="tp")
                nc.tensor.transpose(out=tp2[:], in_=xt2[:].rearrange("p a c -> p (a c)"),
                                    identity=id128[:])
                for a in range(2):
                    i = 2 * i2 + a
                    nc.vector.tensor_copy(out=H[0:C, ts(i, 128)],
                                          in_=tp2[ts(a, 64), :])
                    scr = work.tile([128, C], F32, tag="sqscr")
                    nc.scalar.activation(out=scr[:], in_=xt2[:, a, :],
                                         func=AF.Square,
                                         accum_out=sqmat[:, i:i + 1])
                    pa = psA.tile([128, U], F32, tag="small")
                    nc.tensor.matmul(out=pa[:], lhsT=H[0:C, ts(i, 128)],
                                     rhs=ra16[:, :], start=True, stop=True)
                    if a == 0:
                        asb2 = work.tile([128, 2, U], F32, tag="aev")
                    nc.vector.tensor_copy(out=asb2[:, a, :], in_=pa[:])
                nc.sync.dma_start(
                    out=A_dram[ts(i2, 256), :].rearrange("(a p) u -> p a u", p=128),
                    in_=asb2[:])
            # sq rows: [n_mt, 128] tile-major -> hi/lo fp16 -> flatten into H
            sq_ps = psA.tile([n_mt, 128], F32, tag="small")
            nc.tensor.transpose(out=sq_ps[:], in_=sqmat[:], identity=id128[:])
            sq_sb = big.tile([n_mt, 128], F32)
            nc.scalar.activation(out=sq_sb[:], in_=sq_ps[:], func=AF.Copy,
                                 scale=-0.5)
            sq_h = big.tile([n_mt, 128], F16)
            nc.scalar.copy(out=sq_h[:], in_=sq_sb[:])
            sq_res = big.tile([n_mt, 128], F32)
            nc.vector.tensor_sub(out=sq_res[:], in0=sq_sb[:], in1=sq_h[:])
            sq_l = big.tile([n_mt, 128], F16)
            nc.scalar.copy(out=sq_l[:], in_=sq_res[:])
            nc.sync.dma_start(out=H[64:65, :], in_=sq_h[:])
            nc.sync.dma_start(out=H[65:66, :], in_=sq_l[:])

            # ---------- xrows -> Hr [KC, ROWS] fp16 ----------
            Hr = big.tile([KC, ROWS], F16)
            nc.vector.memset(Hr[64:KC, :], 1.0)
            sqrmat = big.tile([128, n_rt], F32)
            for i2 in range(n_rt // 2):
                xt2 = work.tile([128, 2, C], F32, tag="xload")
                nc.sync.dma_start(
                    out=xt2[:],
                    in_=xrows[ts(i2, 256), :].rearrange("(a p) c -> p a c", p=128),
                )
                tp2 = psT.tile([128, 128], F32, tag="tp")
                nc.tensor.transpose(out=tp2[:], in_=xt2[:].rearrange("p a c -> p (a c)"),
                                    identity=id128[:])
                for a in range(2):
                    i = 2 * i2 + a
                    nc.vector.tensor_copy(out=Hr[0:C, ts(i, 128)],
                                          in_=tp2[ts(a, 64), :])
                    scr = work.tile([128, C], F32, tag="sqscr")
                    nc.scalar.activation(out=scr[:], in_=xt2[:, a, :],
                                         func=AF.Square,
                                         accum_out=sqrmat[:, i:i + 1])
            sqr_ps = psA.tile([n_rt, 128], F32, tag="small")
            nc.tensor.transpose(out=sqr_ps[:], in_=sqrmat[:, 0:n_rt],
                                identity=id128[:])
            # rr = (-1/2-GAMMA)*|x_r|^2 - C0 (centers top-8 scores near 0)
            rr16 = big.tile([n_rt, 128], F16)
            nc.scalar.activation(out=rr16[:], in_=sqr_ps[:], func=AF.Copy,
                                 scale=(-0.5 - GAMMA), bias=-C0)
            nc.sync.dma_start(out=Hr[66:67, :], in_=rr16[:])

            # ---------- main loop over row tiles (software-pipelined) ------
            # (the A_dram barrier is emitted after tile 0's scans so they
            # overlap the preamble tail; gathers only start after it)
            def emit_tail(st):
                gath, t = st
                acc = work.tile([128, U], F32, tag="acc")
                nc.vector.tensor_reduce(
                    out=acc[:], in_=gath[:].rearrange("p k u -> p u k"),
                    axis=mybir.AxisListType.X, op=ALU.max,
                )
                pb = psA.tile([128, U], F32, tag="small")
                nc.tensor.matmul(out=pb[:], lhsT=Hr[:, ts(t, 128)],
                                 rhs=rw16[:, :], start=True, stop=True)
                nc.vector.tensor_add(out=acc[:], in0=acc[:], in1=pb[:])
                ot = work.tile([128, U], F32, tag="ot")
                nc.vector.scalar_tensor_tensor(
                    out=ot[:], in0=acc[:], scalar=NEG_SLOPE, in1=acc[:],
                    op0=ALU.mult, op1=ALU.max,
                )
                nc.sync.dma_start(out=out[ts(t, 128), :], in_=ot[:])

            pending = None
            for t in range(n_rt):
                dist = distp.tile([128, N], F32, tag="dist")
                for j in range(n_ck):
                    pd_ps = psD.tile([128, CHUNK], F32, tag="pd")
                    nc.tensor.matmul(out=pd_ps[:], lhsT=Hr[:, ts(t, 128)],
                                     rhs=H[:, ts(j, CHUNK)], start=True, stop=True)
                    nc.scalar.copy(out=dist[:, ts(j, CHUNK)], in_=pd_ps[:])
                if t == 0:
                    # gathers read A_dram, which Tile does not dep-track.
                    # Probe: a same-ring readback of the last A tile, consumed
                    # by a DVE op, forces the store FIFO to have drained;
                    # the barrier then fences everything before the first
                    # gather, with tile-0's scans (~17us) as extra headroom.
                    probe = work.tile([1, U], F32, tag="probe")
                    nc.sync.dma_start(out=probe[:], in_=A_dram[N - 1:N, :])
                    probe2 = work.tile([1, U], F32, tag="probe2")
                    nc.vector.tensor_copy(out=probe2[:], in_=probe[:])
                    tc.strict_bb_all_engine_barrier()
                vals = work.tile([128, K], F32, tag="vals")
                nc.vector.max(out=vals[:], in_=dist[:])
                idx = work.tile([128, K], U32, tag="idx")
                nc.vector.max_index(out=idx[:], in_max=vals[:], in_values=dist[:])
                if pending is not None:
                    emit_tail(pending)
                gath = work.tile([128, K, U], F32, tag="gath")
                for k2 in range(K):
                    nc.gpsimd.indirect_dma_start(
                        out=gath[:, k2, :], out_offset=None, in_=A_dram[:],
                        in_offset=bass.IndirectOffsetOnAxis(
                            ap=idx[:, k2:k2 + 1], axis=0),
                    )
                pending = (gath, t)
            emit_tail(pending)

    nc.finalize()
    return nc


_NC_CACHE = {}


def _get_nc(N, ROWS, C, U, CHUNK=512):
    key = (N, ROWS, C, U, CHUNK)
    if key not in _NC_CACHE:
        _NC_CACHE[key] = build_nc(N=N, ROWS=ROWS, C=C, U=U, CHUNK=CHUNK)
    return _NC_CACHE[key]


def kernel(inputs, W, bn_gamma, bn_beta, bn_mean, bn_var, _trace=False):
    inputs = np.ascontiguousarray(np.asarray(inputs, dtype=np.float32))
    W = np.ascontiguousarray(np.asarray(W, dtype=np.float32))
    bn_gamma = np.asarray(bn_gamma, dtype=np.float32)
    bn_beta = np.asarray(bn_beta, dtype=np.float32)
    bn_mean = np.asarray(bn_mean, dtype=np.float32)
    bn_var = np.asarray(bn_var, dtype=np.float32)

    B, N, C = inputs.shape
    U = W.shape[0]
    assert B * 2 == N_CORES
    ROWS = N // 2

    nc = _get_nc(N, ROWS, C, U)

    in_maps = []
    for c in range(N_CORES):
        b, hf = c // 2, c % 2
        in_maps.append({
            "xin": inputs[b],
            "xrows": inputs[b, hf * ROWS:(hf + 1) * ROWS],
            "W": W,
            "bn_gamma": bn_gamma,
            "bn_beta": bn_beta,
            "bn_mean": bn_mean,
            "bn_var": bn_var,
        })

    res = run_bass_kernel_spmd(nc, in_maps, list(range(N_CORES)), trace=_trace)

    outp = np.empty((B, N, U), dtype=np.float32)
    for c in range(N_CORES):
        b, hf = c // 2, c % 2
        outp[b, hf * ROWS:(hf + 1) * ROWS] = res.results[c]["out"]
    if _trace:
        return outp, res
    return outp
